# revision 5
# baseline (speedup 1.0000x reference)
"""EuclideanCodebook VQ kernel for 8 Trainium2 NeuronCores (Bass/Tile).

Strategy (data-parallel over tokens, codebook replicated):
  - host: bf16 hi-split of 2x (transposed) and embed (transposed), exact
    fp64->fp32 x2/e2 constants, sharding/layout prep.
  - device per core (8192 tokens): T = 2*x@e.T - e2 via bf16 matmuls into
    PSUM (exact products, ~0.06 abs noise from dropped lo-terms);
    dist = -sqrt(x2 - T) via ACT; top-8 max/argmax via DVE; onehot via
    is_equal(iota, idx); embed_sum/bins via onehot matmul accumulated in
    PSUM; quantize via indirect-DMA gather of fp32 embed rows; AllReduce
    of (embed_sum||bins) across the 8 cores; EMA epilogue on device.
  - host: exact resolution of near-tie argmax tokens (margin on top-2 gap)
    with row-local fixup of quantize/embed_ind/EMA outputs.
"""
import numpy as np
import ml_dtypes

import concourse.bass as bass
import concourse.bacc as bacc
import concourse.mybir as mybir
import concourse.tile as tile
from concourse import bass_utils

F32 = mybir.dt.float32
BF16 = mybir.dt.bfloat16
U32 = mybir.dt.uint32
I32 = mybir.dt.int32
BF = ml_dtypes.bfloat16

NCORES = 8
P = 128
C = 2048          # codebook size
D = 256           # feature dim
NTOT = 65536      # total tokens (32*2048)
NSH = NTOT // NCORES   # 8192 tokens per core
NT = NSH // P          # 64 tiles per core
G = 16                 # tiles per esum group
NGRP = NT // G         # 4 groups
DECAY = 0.8
EPS = 1e-5
MARGIN = 0.5      # host fixup margin on noisy top-2 gap of T
USE_COLLECTIVE = True

_cached = {}


def _build():
    nc = bacc.Bacc("TRN2", target_bir_lowering=False, debug=False,
                   num_devices=NCORES)

    def din(name, shape, dt):
        return nc.dram_tensor(name, shape, dt, kind="ExternalInput").ap()

    def dout(name, shape, dt):
        return nc.dram_tensor(name, shape, dt, kind="ExternalOutput").ap()

    # per-core inputs
    xhT_in = din("xhT", [2, P, NSH], BF16)     # (2x) hi, d-major transposed
    xh_in = din("xh", [NSH, D], BF16)          # x hi, natural (for esum)
    x2_in = din("x2", [P, NT], F32)            # x2[p,t] = |x_{t*128+p}|^2
    ehT_in = din("ehT", [2, P, C], BF16)       # embed hi, transposed
    elT_in = din("elT", [2, P, C], BF16)       # embed lo, transposed
    e2n_in = din("e2n", [3, C], BF16)          # -e2 in 3 bf16 pieces
    emb_in = din("embf", [C, D], F32)          # exact embed (gather source)
    iota_in = din("iotaf", [P, C], F32)
    ident_in = din("identf", [P, P], F32)
    onesb_in = din("onesb", [P, P], BF16)
    onesf_in = din("onesf", [P, P], F32)
    cs_pm_in = din("cspm", [P, 16], F32)       # cluster_size, c = 16p+j
    cs_cd_in = din("cscd", [P, 16], F32)       # cluster_size, c = 128j+p
    eavg_in = din("eavg", [C, D], F32)

    # per-core outputs
    dist_out = dout("dist_o", [NSH, C], F32)
    quant_out = dout("quant_o", [NSH, D], F32)
    ind_out = dout("ind_o", [NT, P], I32)      # ind[t, p] = token t*128+p
    m8_out = dout("m8_o", [P, NT * 8], F32)
    idx8_out = dout("idx8_o", [P, NT * 8], U32)
    esumred_out = dout("esr_o", [257, C], F32)  # reduced (esumT || bins)
    ncs_out = dout("ncs_o", [P, 16], F32)       # new_cluster_size, c=16p+j
    nea_out = dout("nea_o", [C, D], F32)        # new_embed_avg
    ne_out = dout("ne_o", [C, D], F32)          # new_embed

    with tile.TileContext(nc) as tc:
        with (
            tc.tile_pool(name="const", bufs=1) as cp,
            tc.tile_pool(name="work", bufs=1) as wp,
            tc.tile_pool(name="ps", bufs=2, space="PSUM") as pp,
            tc.tile_pool(name="dram", bufs=1, space="DRAM") as dp,
        ):
            # ---------------- constants ----------------
            ehT = cp.tile([P, 2, C], BF16, tag="ehT")
            for k in range(2):
                nc.sync.dma_start(ehT[:, k, :], ehT_in[k])
            e2n = cp.tile([P, C], BF16, tag="e2n")
            nc.sync.dma_start(e2n[0:3, :], e2n_in[:])
            iota_t = cp.tile([P, C], F32, tag="iota")
            nc.sync.dma_start(iota_t[:], iota_in[:])
            ident = cp.tile([P, P], F32, tag="ident")
            nc.sync.dma_start(ident[:], ident_in[:])
            onesb = cp.tile([P, P], BF16, tag="onesb")
            nc.sync.dma_start(onesb[:], onesb_in[:])
            onesf = cp.tile([P, P], F32, tag="onesf")
            nc.sync.dma_start(onesf[:], onesf_in[:])
            x2sb = cp.tile([P, NT], F32, tag="x2sb")
            nc.sync.dma_start(x2sb[:], x2_in[:])

            m8st = cp.tile([P, NT * 8], F32, tag="m8st")
            idx8st = cp.tile([P, NT * 8], U32, tag="idx8st")
            idxf = cp.tile([P, NT], F32, tag="idxf")
            esacc = cp.tile([P, 3, C], F32, tag="esacc")
            nc.vector.memset(esacc[:], 0.0)

            # ---------------- main loop ----------------
            for g in range(NGRP):
                tiles = range(g * G, (g + 1) * G)
                o_tiles = {}
                xh_tiles = {}
                for t in tiles:
                    # loads
                    aht = wp.tile([P, 2, P], BF16, tag="aht", bufs=3)
                    for k in range(2):
                        nc.sync.dma_start(
                            aht[:, k, :], xhT_in[k][:, t * P:(t + 1) * P])
                    xh_t = wp.tile([P, D], BF16, tag="xh", bufs=G + 2)
                    nc.sync.dma_start(xh_t[:], xh_in[t * P:(t + 1) * P, :])
                    xh_tiles[t] = xh_t

                    # T = 2 x e^T - e2  (bf16 hi products only)
                    ps_t = pp.tile([P, C], F32, tag="ps")
                    for k in range(2):
                        for q in range(4):
                            nc.tensor.matmul(
                                ps_t[:, q * 512:(q + 1) * 512],
                                lhsT=aht[:, k, :],
                                rhs=ehT[:, k, q * 512:(q + 1) * 512],
                                start=(k == 0), stop=False)
                    for q in range(4):
                        nc.tensor.matmul(
                            ps_t[:, q * 512:(q + 1) * 512],
                            lhsT=onesb[0:3, :],
                            rhs=e2n[0:3, q * 512:(q + 1) * 512],
                            start=False, stop=(q == 3))

                    # dist = -sqrt(x2 - T)
                    d_t = wp.tile([P, C], F32, tag="dist", bufs=3)
                    nc.scalar.activation(
                        d_t[:], ps_t[:], mybir.ActivationFunctionType.Sqrt,
                        bias=x2sb[:, t:t + 1], scale=-1.0)
                    nc.scalar.mul(d_t[:], d_t[:], -1.0)
                    nc.sync.dma_start(dist_out[t * P:(t + 1) * P, :], d_t[:])

                    # top-8 + indices from PSUM
                    m8 = m8st[:, t * 8:(t + 1) * 8]
                    nc.vector.max(m8, ps_t[:])
                    i8 = idx8st[:, t * 8:(t + 1) * 8]
                    nc.vector.max_index(i8, m8, ps_t[:])

                    # idx as f32 (exact for < 2^24)
                    nc.vector.tensor_copy(idxf[:, t:t + 1],
                                          idx8st[:, t * 8:t * 8 + 1])
                    # onehot
                    o_t = wp.tile([P, C], BF16, tag="oh", bufs=G + 2)
                    nc.vector.tensor_scalar(
                        out=o_t[:], in0=iota_t[:],
                        scalar1=idxf[:, t:t + 1], scalar2=None,
                        op0=mybir.AluOpType.is_equal)
                    o_tiles[t] = o_t

                    # quantize gather (exact fp32 embed rows)
                    q_t = wp.tile([P, D], F32, tag="qt", bufs=3)
                    nc.gpsimd.indirect_dma_start(
                        out=q_t[:], out_offset=None, in_=emb_in[:],
                        in_offset=bass.IndirectOffsetOnAxis(
                            ap=idx8st[:, t * 8:t * 8 + 1], axis=0))
                    nc.sync.dma_start(quant_out[t * P:(t + 1) * P, :], q_t[:])

                # ---- esum phase for this group ----
                for cch in range(3):
                    ps_e = pp.tile([P, C], F32, tag="ps")
                    mrows = P if cch < 2 else 1
                    for i, t in enumerate(tiles):
                        if cch < 2:
                            lhsT = xh_tiles[t][:, cch * P:(cch + 1) * P]
                        else:
                            lhsT = onesb[:, 0:1]
                        for q in range(4):
                            nc.tensor.matmul(
                                ps_e[0:mrows, q * 512:(q + 1) * 512],
                                lhsT=lhsT,
                                rhs=o_tiles[t][:, q * 512:(q + 1) * 512],
                                start=(i == 0), stop=(i == G - 1))
                    if cch < 2:
                        nc.vector.tensor_tensor(
                            out=esacc[:, cch, :], in0=esacc[:, cch, :],
                            in1=ps_e[:], op=mybir.AluOpType.add)
                    else:
                        nc.vector.tensor_tensor(
                            out=esacc[0:1, 2, :], in0=esacc[0:1, 2, :],
                            in1=ps_e[0:1, :], op=mybir.AluOpType.add)

            # ---------------- embed_ind output ----------------
            ps_i = pp.tile([P, C], F32, tag="ps")
            nc.tensor.transpose(ps_i[0:NT, 0:P], idxf[:, 0:NT], ident[:])
            ind_sb = wp.tile([NT, P], I32, tag="indsb")
            nc.vector.tensor_copy(ind_sb[:], ps_i[0:NT, 0:P])
            nc.sync.dma_start(ind_out[:], ind_sb[:])
            nc.sync.dma_start(m8_out[:], m8st[:])
            nc.sync.dma_start(idx8_out[:], idx8st[:])

            # ---------------- all-reduce esum ----------------
            part = dp.tile([257, C], F32)
            nc.sync.dma_start(part[0:P, :], esacc[:, 0, :])
            nc.sync.dma_start(part[P:2 * P, :], esacc[:, 1, :])
            nc.sync.dma_start(part[2 * P:257, :], esacc[0:1, 2, :])
            if USE_COLLECTIVE:
                red = dp.tile([257, C], F32, addr_space="Shared")
                nc.gpsimd.collective_compute(
                    "AllReduce", mybir.AluOpType.add,
                    replica_groups=[list(range(NCORES))],
                    ins=[part.opt()], outs=[red.opt()])
            else:
                red = part
            nc.sync.dma_start(esumred_out[:], red[:])

            # load reduced back
            redT = wp.tile([P, 2, C], F32, tag="redT")
            nc.sync.dma_start(redT[:, 0, :], red[0:P, :])
            nc.sync.dma_start(redT[:, 1, :], red[P:2 * P, :])
            binsrow = wp.tile([P, C], F32, tag="binsrow")
            nc.sync.dma_start(binsrow[0:1, :], red[2 * P:257, :])
            binspm = wp.tile([P, 16], F32, tag="binspm")
            nc.sync.dma_start(
                binspm[:],
                red[2 * P:257, :].rearrange("one (p j) -> (one p) j", p=P))

            cs_pm = wp.tile([P, 16], F32, tag="cspm")
            nc.sync.dma_start(cs_pm[:], cs_pm_in[:])
            cs_cd = wp.tile([P, 16], F32, tag="cscd")
            nc.sync.dma_start(cs_cd[:], cs_cd_in[:])

            # bins in code-tile layout via 16 mini transposes
            binscd = wp.tile([P, 16], F32, tag="binscd")
            for i in range(16):
                ps_b = pp.tile([P, C], F32, tag="ps")
                nc.tensor.transpose(
                    ps_b[0:P, 0:1], binsrow[0:1, i * P:(i + 1) * P],
                    ident[0:1, 0:1])
                nc.scalar.copy(binscd[:, i:i + 1], ps_b[0:P, 0:1])

            # ncs (both layouts)
            ncs_pm = wp.tile([P, 16], F32, tag="ncspm")
            nc.vector.tensor_scalar(out=ncs_pm[:], in0=cs_pm[:],
                                    scalar1=DECAY, scalar2=None,
                                    op0=mybir.AluOpType.mult)
            tmp_pm = wp.tile([P, 16], F32, tag="tmppm")
            nc.vector.tensor_scalar(out=tmp_pm[:], in0=binspm[:],
                                    scalar1=1.0 - DECAY, scalar2=None,
                                    op0=mybir.AluOpType.mult)
            nc.vector.tensor_tensor(out=ncs_pm[:], in0=ncs_pm[:],
                                    in1=tmp_pm[:], op=mybir.AluOpType.add)
            nc.sync.dma_start(ncs_out[:], ncs_pm[:])

            ncs_cd = wp.tile([P, 16], F32, tag="ncscd")
            nc.vector.tensor_scalar(out=ncs_cd[:], in0=cs_cd[:],
                                    scalar1=DECAY, scalar2=None,
                                    op0=mybir.AluOpType.mult)
            tmp_cd = wp.tile([P, 16], F32, tag="tmpcd")
            nc.vector.tensor_scalar(out=tmp_cd[:], in0=binscd[:],
                                    scalar1=1.0 - DECAY, scalar2=None,
                                    op0=mybir.AluOpType.mult)
            nc.vector.tensor_tensor(out=ncs_cd[:], in0=ncs_cd[:],
                                    in1=tmp_cd[:], op=mybir.AluOpType.add)

            # total = sum(ncs); r = total / (total + C*EPS)
            rowsum = wp.tile([P, 1], F32, tag="rowsum")
            nc.vector.tensor_reduce(rowsum[:], ncs_pm[:],
                                    axis=mybir.AxisListType.X,
                                    op=mybir.AluOpType.add)
            ps_s = pp.tile([P, C], F32, tag="ps")
            nc.tensor.matmul(ps_s[0:1, 0:1], lhsT=onesf[:, 0:1],
                             rhs=rowsum[:], start=True, stop=True)
            t11 = wp.tile([P, 1], F32, tag="t11")
            nc.scalar.copy(t11[0:1, :], ps_s[0:1, 0:1])
            ps_bc = pp.tile([P, C], F32, tag="ps")
            nc.tensor.matmul(ps_bc[0:P, 0:1], lhsT=onesf[0:1, :],
                             rhs=t11[0:1, 0:1], start=True, stop=True)
            totb = wp.tile([P, 1], F32, tag="totb")
            nc.scalar.copy(totb[:], ps_bc[0:P, 0:1])
            tot_eps = wp.tile([P, 1], F32, tag="toteps")
            nc.vector.tensor_scalar(out=tot_eps[:], in0=totb[:],
                                    scalar1=float(C) * EPS, scalar2=None,
                                    op0=mybir.AluOpType.add)
            rinv = wp.tile([P, 1], F32, tag="rinv")
            nc.vector.reciprocal(rinv[:], tot_eps[:])
            rfac = wp.tile([P, 1], F32, tag="rfac")
            nc.vector.tensor_tensor(out=rfac[:], in0=totb[:], in1=rinv[:],
                                    op=mybir.AluOpType.mult)

            # smoothed & reciprocal (code-tile layout)
            sm_cd = wp.tile([P, 16], F32, tag="smcd")
            nc.vector.tensor_scalar(out=sm_cd[:], in0=ncs_cd[:],
                                    scalar1=EPS, scalar2=rfac[:],
                                    op0=mybir.AluOpType.add,
                                    op1=mybir.AluOpType.mult)
            rsm_cd = wp.tile([P, 16], F32, tag="rsmcd")
            nc.vector.reciprocal(rsm_cd[:], sm_cd[:])

            # per code-tile EMA
            for i in range(16):
                es_cd = wp.tile([P, D], F32, tag="escd", bufs=2)
                for k in range(2):
                    ps_r = pp.tile([P, C], F32, tag="ps")
                    nc.tensor.transpose(
                        ps_r[0:P, 0:P],
                        redT[:, k, i * P:(i + 1) * P], ident[:])
                    nc.scalar.copy(es_cd[:, k * P:(k + 1) * P],
                                   ps_r[0:P, 0:P])
                eavg_i = wp.tile([P, D], F32, tag="eavgi", bufs=2)
                nc.sync.dma_start(eavg_i[:], eavg_in[i * P:(i + 1) * P, :])
                nea_i = wp.tile([P, D], F32, tag="neai", bufs=2)
                nc.vector.tensor_scalar(out=nea_i[:], in0=eavg_i[:],
                                        scalar1=DECAY, scalar2=None,
                                        op0=mybir.AluOpType.mult)
                tmp_i = wp.tile([P, D], F32, tag="tmpi", bufs=2)
                nc.vector.tensor_scalar(out=tmp_i[:], in0=es_cd[:],
                                        scalar1=1.0 - DECAY, scalar2=None,
                                        op0=mybir.AluOpType.mult)
                nc.vector.tensor_tensor(out=nea_i[:], in0=nea_i[:],
                                        in1=tmp_i[:], op=mybir.AluOpType.add)
                nc.sync.dma_start(nea_out[i * P:(i + 1) * P, :], nea_i[:])
                ne_i = wp.tile([P, D], F32, tag="nei", bufs=2)
                nc.vector.tensor_scalar(out=ne_i[:], in0=nea_i[:],
                                        scalar1=rsm_cd[:, i:i + 1],
                                        scalar2=None,
                                        op0=mybir.AluOpType.mult)
                nc.sync.dma_start(ne_out[i * P:(i + 1) * P, :], ne_i[:])

    nc.compile()
    return nc


def _prep_inputs(x, embed, embed_avg, cluster_size):
    """Host-side layout/precision prep. Returns per-core in_maps."""
    x = np.asarray(x, np.float32).reshape(NTOT, D)
    embed = np.asarray(embed, np.float32).reshape(C, D)
    embed_avg = np.asarray(embed_avg, np.float32).reshape(C, D)
    cluster_size = np.asarray(cluster_size, np.float32).reshape(C)

    a = 2.0 * x                                  # exact
    ah = a.astype(BF)
    e2 = (embed.astype(np.float64) ** 2).sum(-1)
    x2 = (x.astype(np.float64) ** 2).sum(-1).astype(np.float32)

    eh = embed.astype(BF)
    el = (embed - eh.astype(np.float32)).astype(BF)
    ehT = np.ascontiguousarray(eh.astype(np.float32).T.astype(BF)
                               .reshape(2, P, C))
    elT = np.ascontiguousarray(el.astype(np.float32).T.astype(BF)
                               .reshape(2, P, C))

    e2n = np.zeros((3, C), np.float32)
    r = -e2.copy()
    for i in range(3):
        p = r.astype(np.float32).astype(BF).astype(np.float32)
        e2n[i] = p
        r = r - p
    e2n = e2n.astype(BF)

    iota = np.broadcast_to(np.arange(C, dtype=np.float32), (P, C)).copy()
    ident = np.eye(P, dtype=np.float32)
    onesb = np.ones((P, P), BF)
    onesf = np.ones((P, P), np.float32)
    cs_pm = cluster_size.reshape(P, 16).copy()
    cs_cd = np.ascontiguousarray(cluster_size.reshape(16, P).T)

    shared = {
        "ehT": ehT, "elT": elT, "e2n": e2n, "embf": embed,
        "iotaf": iota, "identf": ident, "onesb": onesb, "onesf": onesf,
        "cspm": cs_pm, "cscd": cs_cd, "eavg": embed_avg,
    }
    in_maps = []
    for c in range(NCORES):
        sl = slice(c * NSH, (c + 1) * NSH)
        ah_c = ah[sl]                            # (8192, 256) bf16
        xhT = np.ascontiguousarray(
            ah_c.astype(np.float32).T.astype(BF).reshape(2, P, NSH))
        xh = x[sl].astype(BF)
        x2_c = np.ascontiguousarray(x2[sl].reshape(NT, P).T)
        m = dict(shared)
        m.update({"xhT": xhT, "xh": xh, "x2": x2_c})
        in_maps.append(m)
    return in_maps


def _host_fixup(x, embed, cluster_size, embed_avg, outs):
    """Resolve near-tie argmax tokens exactly; patch outputs in place."""
    x64 = np.asarray(x, np.float64).reshape(NTOT, D)
    e64 = np.asarray(embed, np.float64).reshape(C, D)
    embed = np.asarray(embed, np.float32).reshape(C, D)
    e2_64 = (e64 ** 2).sum(-1)

    ind = outs["embed_ind"]
    m8 = outs["m8"]
    idx8 = outs["idx8"]
    gaps = m8[:, 0] - m8[:, 1]
    amb = np.nonzero(gaps < MARGIN)[0]
    flips = []
    for n in amb:
        cands = idx8[n].astype(np.int64)
        xv = x64[n]
        sq = e2_64[cands] - 2.0 * (e64[cands] @ xv)
        best = int(cands[np.argmin(sq)])
        if best != int(ind[n]):
            flips.append((int(n), int(ind[n]), best))
            ind[n] = best
            outs["quantize"][n] = embed[best]

    if flips:
        cs = np.asarray(cluster_size, np.float32).reshape(C)
        eavg = np.asarray(embed_avg, np.float32).reshape(C, D)
        esum = outs["esum_red"][0:2 * P, :]      # (256, 2048) d-major
        bins = outs["esum_red"][2 * P, :].copy()  # (2048,)
        esum_cd = np.ascontiguousarray(esum.T)    # (2048, 256)
        xh32 = np.asarray(x, np.float32).reshape(NTOT, D).astype(BF)\
            .astype(np.float32)
        for n, old, new in flips:
            bins[old] -= 1.0
            bins[new] += 1.0
            esum_cd[old] -= xh32[n]
            esum_cd[new] += xh32[n]
        ncs_full = outs["new_cluster_size"]
        total = np.float32(ncs_full.sum(dtype=np.float32))
        rfac = np.float32(total / (total + np.float32(C * EPS)))
        affected = sorted({cd for f in flips for cd in f[1:]})
        for cd in affected:
            ncs = np.float32(DECAY) * cs[cd] + np.float32(1 - DECAY) * bins[cd]
            ncs_full[cd] = ncs
            nea = (np.float32(DECAY) * eavg[cd]
                   + np.float32(1 - DECAY) * esum_cd[cd])
            outs["new_embed_avg"][cd] = nea
            sm = (ncs + np.float32(EPS)) * rfac
            outs["new_embed"][cd] = nea * (np.float32(1.0) / sm)
    outs["n_ambiguous"] = len(amb)
    outs["n_flips"] = len(flips)
    return outs


_last_exec_ns = None
_last_profile = None


def kernel(x, embed, embed_avg, cluster_size):
    global _last_exec_ns, _last_profile
    import os
    if "nc" not in _cached:
        _cached["nc"] = _build()
    nc = _cached["nc"]
    in_maps = _prep_inputs(x, embed, embed_avg, cluster_size)
    trace = bool(os.environ.get("VQ_TRACE"))
    res = bass_utils.run_bass_kernel_spmd(
        nc, in_maps, core_ids=list(range(NCORES)), trace=trace)
    _last_exec_ns = res.exec_time_ns
    _last_profile = res
    rs = res.results

    dist = np.concatenate([r["dist_o"] for r in rs], 0).reshape(1, NTOT, C)
    quant = np.concatenate([r["quant_o"] for r in rs], 0)
    ind = np.concatenate([r["ind_o"].reshape(NSH) for r in rs], 0)\
        .astype(np.int32)
    m8 = np.concatenate(
        [r["m8_o"].reshape(P, NT, 8).transpose(1, 0, 2).reshape(NSH, 8)
         for r in rs], 0)
    idx8 = np.concatenate(
        [r["idx8_o"].reshape(P, NT, 8).transpose(1, 0, 2).reshape(NSH, 8)
         for r in rs], 0)
    r0 = rs[0]
    outs = {
        "quantize": quant,
        "embed_ind": ind,
        "dist": dist,
        "new_embed": r0["ne_o"].copy(),
        "new_cluster_size": r0["ncs_o"].reshape(C).copy(),
        "new_embed_avg": r0["nea_o"].copy(),
        "esum_red": r0["esr_o"],
        "m8": m8,
        "idx8": idx8,
    }
    _host_fixup(x, embed, cluster_size, embed_avg, outs)

    h, b, n, d = 1, 32, 2048, 256
    return (
        outs["quantize"].reshape(h, b, n, d),
        outs["embed_ind"].reshape(h, b, n),
        outs["dist"],
        outs["new_embed"].reshape(h, C, D),
        outs["new_cluster_size"].reshape(h, C),
        outs["new_embed_avg"].reshape(h, C, D),
    )


# revision 13
# speedup vs baseline: 108.9664x; 108.9664x over previous
"""EuclideanCodebook VQ kernel for 8 Trainium2 NeuronCores (Bass/Tile).

Strategy (data-parallel over tokens, codebook replicated):
  - host: bf16 hi-split of 2x (transposed) and embed (transposed), exact
    fp64->fp32 x2/e2 constants, sharding/layout prep.
  - device per core (8192 tokens): T = 2*x@e.T - e2 via bf16 matmuls into
    PSUM (exact products, ~0.06 abs noise from dropped lo-terms);
    dist = -sqrt(x2 - T) via ACT; top-8 max/argmax via DVE; onehot via
    is_equal(iota, idx); embed_sum/bins via onehot matmul accumulated in
    PSUM; quantize via indirect-DMA gather of fp32 embed rows; AllReduce
    of (embed_sum||bins) across the 8 cores; EMA epilogue on device.
  - host: exact resolution of near-tie argmax tokens (margin on top-2 gap)
    with row-local fixup of quantize/embed_ind/EMA outputs.
"""
import numpy as np
import ml_dtypes

import concourse.bass as bass
import concourse.bacc as bacc
import concourse.mybir as mybir
import concourse.tile as tile
from concourse import bass_utils

F32 = mybir.dt.float32
BF16 = mybir.dt.bfloat16
U32 = mybir.dt.uint32
I32 = mybir.dt.int32
BF = ml_dtypes.bfloat16

NCORES = 8
P = 128
C = 2048          # codebook size
D = 256           # feature dim
NTOT = 65536      # total tokens (32*2048)
NSH = NTOT // NCORES   # 8192 tokens per core
NT = NSH // P          # 64 tiles per core
G = 8                  # tiles per esum group
NGRP = NT // G         # 4 groups
DECAY = 0.8
EPS = 1e-5
MARGIN = 0.02     # host fixup margin on noisy top-2 dist gap
USE_COLLECTIVE = True

_cached = {}


def _build(ablate=()):
    ab = set(ablate)
    nc = bacc.Bacc("TRN2", target_bir_lowering=False, debug=False,
                   num_devices=NCORES)

    def din(name, shape, dt):
        return nc.dram_tensor(name, shape, dt, kind="ExternalInput").ap()

    def dout(name, shape, dt):
        return nc.dram_tensor(name, shape, dt, kind="ExternalOutput").ap()

    # per-core inputs
    xhT_in = din("xhT", [2, P, NSH], BF16)     # (2x) hi, d-major transposed
    xh_in = din("xh", [NSH, D], BF16)          # x hi, natural (for esum)
    x2_in = din("x2", [P, NT], F32)            # x2[p,t] = |x_{t*128+p}|^2
    ehT_in = din("ehT", [2, P, C], BF16)       # embed hi, transposed
    elT_in = din("elT", [2, P, C], BF16)       # embed lo, transposed
    e2n_in = din("e2n", [3, C], BF16)          # -e2 in 3 bf16 pieces
    emb_in = din("embf", [C, D], F32)          # exact embed (gather source)
    iota_in = din("iotaf", [P, C], F32)
    ident_in = din("identf", [P, P], F32)
    onesb_in = din("onesb", [P, P], BF16)
    onesf_in = din("onesf", [P, P], F32)
    cs_pm_in = din("cspm", [P, 16], F32)       # cluster_size, c = 16p+j
    cs_cd_in = din("cscd", [P, 16], F32)       # cluster_size, c = 128j+p
    eavg_in = din("eavg", [C, D], F32)

    # per-core outputs
    dist_out = dout("dist_o", [NSH, C], F32)
    quant_out = dout("quant_o", [NSH, D], F32)
    ind_out = dout("ind_o", [NT, P], I32)      # ind[t, p] = token t*128+p
    m8_out = dout("m8_o", [P, NT * 8], F32)
    idx8_out = dout("idx8_o", [P, NT * 8], U32)
    esumred_out = dout("esr_o", [257, C], F32)  # reduced (esumT || bins)
    ncs_out = dout("ncs_o", [P, 16], F32)       # new_cluster_size, c=16p+j
    nea_out = dout("nea_o", [C, D], F32)        # new_embed_avg
    ne_out = dout("ne_o", [C, D], F32)          # new_embed

    with tile.TileContext(nc) as tc:
        with (
            tc.tile_pool(name="const", bufs=1) as cp,
            tc.tile_pool(name="work", bufs=1) as wp,
            tc.tile_pool(name="ps", bufs=2, space="PSUM") as pp,
            tc.tile_pool(name="dram", bufs=1, space="DRAM") as dp,
        ):
            # ---------------- constants ----------------
            ehT = cp.tile([P, 2, C], BF16, tag="ehT")
            for k in range(2):
                nc.sync.dma_start(ehT[:, k, :], ehT_in[k])
            e2n = cp.tile([P, C], BF16, tag="e2n")
            nc.sync.dma_start(e2n[0:3, :], e2n_in[:])
            iota_t = cp.tile([P, C], F32, tag="iota")
            nc.sync.dma_start(iota_t[:], iota_in[:])
            ident = cp.tile([P, P], F32, tag="ident")
            nc.sync.dma_start(ident[:], ident_in[:])
            onesb = cp.tile([P, P], BF16, tag="onesb")
            nc.sync.dma_start(onesb[:], onesb_in[:])
            onesf = cp.tile([P, P], F32, tag="onesf")
            nc.sync.dma_start(onesf[:], onesf_in[:])
            x2sb = cp.tile([P, NT], F32, tag="x2sb")
            nc.sync.dma_start(x2sb[:], x2_in[:])

            m8st = cp.tile([P, NT * 8], F32, tag="m8st")
            idx8st = cp.tile([P, NT * 8], U32, tag="idx8st")
            idxf = cp.tile([P, NT], F32, tag="idxf")
            esacc = cp.tile([P, 3, C], F32, tag="esacc")
            nc.vector.memset(esacc[:], 0.0)

            # ---------------- main loop ----------------
            for g in range(NGRP):
                tiles = range(g * G, (g + 1) * G)
                o_tiles = {}
                xh_tiles = {}
                for t in tiles:
                    # loads
                    aht = wp.tile([P, 2, P], BF16, tag="aht", bufs=6)
                    for k in range(2):
                        nc.sync.dma_start(
                            aht[:, k, :], xhT_in[k][:, t * P:(t + 1) * P])
                    xh_t = wp.tile([P, D], BF16, tag="xh", bufs=G + 2)
                    nc.sync.dma_start(xh_t[:], xh_in[t * P:(t + 1) * P, :])
                    xh_tiles[t] = xh_t

                    # T = 2 x e^T - e2  (bf16 hi products only)
                    ps_t = pp.tile([P, C], F32, tag="ps")
                    if "mm" not in ab:
                        for k in range(2):
                            for q in range(4):
                                nc.tensor.matmul(
                                    ps_t[:, q * 512:(q + 1) * 512],
                                    lhsT=aht[:, k, :],
                                    rhs=ehT[:, k, q * 512:(q + 1) * 512],
                                    start=(k == 0), stop=False)
                        for q in range(4):
                            nc.tensor.matmul(
                                ps_t[:, q * 512:(q + 1) * 512],
                                lhsT=onesb[0:3, :],
                                rhs=e2n[0:3, q * 512:(q + 1) * 512],
                                start=False, stop=(q == 3))
                    else:
                        nc.vector.memset(ps_t[:], 0.0)

                    # dist = -sqrt(x2 - T)
                    d_t = wp.tile([P, C], F32, tag="dist", bufs=6)
                    if "act" not in ab:
                        nc.scalar.activation(
                            d_t[:], ps_t[:],
                            mybir.ActivationFunctionType.Sqrt,
                            bias=x2sb[:, t:t + 1], scale=-1.0)
                        nc.scalar.mul(d_t[:], d_t[:], -1.0)
                    if "distdma" not in ab and "act" not in ab:
                        nc.scalar.dma_start(
                            dist_out[t * P:(t + 1) * P, :], d_t[:])

                    # top-8 + indices from the (negated) dist tile in SBUF
                    if "argmax" not in ab:
                        m8 = m8st[:, t * 8:(t + 1) * 8]
                        nc.vector.max(m8, ps_t[:] if "psarg" in ab else d_t[:])
                        i8 = idx8st[:, t * 8:(t + 1) * 8]
                        nc.vector.max_index(i8, m8, ps_t[:] if "psarg" in ab else d_t[:])
                        nc.vector.tensor_copy(idxf[:, t:t + 1],
                                              idx8st[:, t * 8:t * 8 + 1])
                    # onehot
                    o_t = wp.tile([P, C], BF16, tag="oh", bufs=G + 2)
                    if "onehot" not in ab:
                        eng = nc.vector if "ohdve" in ab else nc.gpsimd
                        eng.tensor_scalar(
                            out=o_t[:], in0=iota_t[:],
                            scalar1=idxf[:, t:t + 1], scalar2=None,
                            op0=mybir.AluOpType.is_equal)
                    o_tiles[t] = o_t

                    # quantize gather (exact fp32 embed rows)
                    if "gather" not in ab:
                        q_t = wp.tile([P, D], F32, tag="qt", bufs=3)
                        nc.gpsimd.indirect_dma_start(
                            out=q_t[:], out_offset=None, in_=emb_in[:],
                            in_offset=bass.IndirectOffsetOnAxis(
                                ap=idx8st[:, t * 8:t * 8 + 1], axis=0))
                        nc.sync.dma_start(
                            quant_out[t * P:(t + 1) * P, :], q_t[:])

                # ---- esum phase for this group ----
                for cch in (() if "esum" in ab else range(3)):
                    ps_e = pp.tile([P, C], F32, tag="ps")
                    mrows = P if cch < 2 else 1
                    for i, t in enumerate(tiles):
                        if cch < 2:
                            lhsT = xh_tiles[t][:, cch * P:(cch + 1) * P]
                        else:
                            lhsT = onesb[:, 0:1]
                        for q in range(4):
                            nc.tensor.matmul(
                                ps_e[0:mrows, q * 512:(q + 1) * 512],
                                lhsT=lhsT,
                                rhs=o_tiles[t][:, q * 512:(q + 1) * 512],
                                start=(i == 0), stop=(i == G - 1))
                    if cch < 2:
                        nc.vector.tensor_tensor(
                            out=esacc[:, cch, :], in0=esacc[:, cch, :],
                            in1=ps_e[:], op=mybir.AluOpType.add)
                    else:
                        nc.vector.tensor_tensor(
                            out=esacc[0:1, 2, :], in0=esacc[0:1, 2, :],
                            in1=ps_e[0:1, :], op=mybir.AluOpType.add)

            # ---------------- embed_ind output ----------------
            ps_i = pp.tile([P, C], F32, tag="ps")
            nc.tensor.transpose(ps_i[0:NT, 0:P], idxf[:, 0:NT], ident[:])
            ind_sb = wp.tile([NT, P], I32, tag="indsb")
            nc.vector.tensor_copy(ind_sb[:], ps_i[0:NT, 0:P])
            nc.sync.dma_start(ind_out[:], ind_sb[:])
            nc.sync.dma_start(m8_out[:], m8st[:])
            nc.sync.dma_start(idx8_out[:], idx8st[:])

            # ---------------- all-reduce esum ----------------
            part = dp.tile([257, C], F32)
            nc.sync.dma_start(part[0:P, :], esacc[:, 0, :])
            nc.sync.dma_start(part[P:2 * P, :], esacc[:, 1, :])
            nc.sync.dma_start(part[2 * P:257, :], esacc[0:1, 2, :])
            if USE_COLLECTIVE:
                red = dp.tile([257, C], F32, addr_space="Shared")
                nc.gpsimd.collective_compute(
                    "AllReduce", mybir.AluOpType.add,
                    replica_groups=[list(range(NCORES))],
                    ins=[part.opt()], outs=[red.opt()])
            else:
                red = part
            nc.sync.dma_start(esumred_out[:], red[:])

            # load reduced back
            redT = wp.tile([P, 2, C], F32, tag="redT")
            nc.sync.dma_start(redT[:, 0, :], red[0:P, :])
            nc.sync.dma_start(redT[:, 1, :], red[P:2 * P, :])
            binsrow = wp.tile([P, C], F32, tag="binsrow")
            nc.sync.dma_start(binsrow[0:1, :], red[2 * P:257, :])
            binspm = wp.tile([P, 16], F32, tag="binspm")
            nc.sync.dma_start(
                binspm[:],
                red[2 * P:257, :].rearrange("one (p j) -> (one p) j", p=P))

            cs_pm = wp.tile([P, 16], F32, tag="cspm")
            nc.sync.dma_start(cs_pm[:], cs_pm_in[:])
            cs_cd = wp.tile([P, 16], F32, tag="cscd")
            nc.sync.dma_start(cs_cd[:], cs_cd_in[:])

            # bins in code-tile layout via 16 mini transposes
            binscd = wp.tile([P, 16], F32, tag="binscd")
            for i in range(16):
                ps_b = pp.tile([P, C], F32, tag="ps")
                nc.tensor.transpose(
                    ps_b[0:P, 0:1], binsrow[0:1, i * P:(i + 1) * P],
                    ident[0:1, 0:1])
                nc.scalar.copy(binscd[:, i:i + 1], ps_b[0:P, 0:1])

            # ncs (both layouts)
            ncs_pm = wp.tile([P, 16], F32, tag="ncspm")
            nc.vector.tensor_scalar(out=ncs_pm[:], in0=cs_pm[:],
                                    scalar1=DECAY, scalar2=None,
                                    op0=mybir.AluOpType.mult)
            tmp_pm = wp.tile([P, 16], F32, tag="tmppm")
            nc.vector.tensor_scalar(out=tmp_pm[:], in0=binspm[:],
                                    scalar1=1.0 - DECAY, scalar2=None,
                                    op0=mybir.AluOpType.mult)
            nc.vector.tensor_tensor(out=ncs_pm[:], in0=ncs_pm[:],
                                    in1=tmp_pm[:], op=mybir.AluOpType.add)
            nc.sync.dma_start(ncs_out[:], ncs_pm[:])

            ncs_cd = wp.tile([P, 16], F32, tag="ncscd")
            nc.vector.tensor_scalar(out=ncs_cd[:], in0=cs_cd[:],
                                    scalar1=DECAY, scalar2=None,
                                    op0=mybir.AluOpType.mult)
            tmp_cd = wp.tile([P, 16], F32, tag="tmpcd")
            nc.vector.tensor_scalar(out=tmp_cd[:], in0=binscd[:],
                                    scalar1=1.0 - DECAY, scalar2=None,
                                    op0=mybir.AluOpType.mult)
            nc.vector.tensor_tensor(out=ncs_cd[:], in0=ncs_cd[:],
                                    in1=tmp_cd[:], op=mybir.AluOpType.add)

            # total = sum(ncs); r = total / (total + C*EPS)
            rowsum = wp.tile([P, 1], F32, tag="rowsum")
            nc.vector.tensor_reduce(rowsum[:], ncs_pm[:],
                                    axis=mybir.AxisListType.X,
                                    op=mybir.AluOpType.add)
            ps_s = pp.tile([P, C], F32, tag="ps")
            nc.tensor.matmul(ps_s[0:1, 0:1], lhsT=onesf[:, 0:1],
                             rhs=rowsum[:], start=True, stop=True)
            t11 = wp.tile([P, 1], F32, tag="t11")
            nc.scalar.copy(t11[0:1, :], ps_s[0:1, 0:1])
            ps_bc = pp.tile([P, C], F32, tag="ps")
            nc.tensor.matmul(ps_bc[0:P, 0:1], lhsT=onesf[0:1, :],
                             rhs=t11[0:1, 0:1], start=True, stop=True)
            totb = wp.tile([P, 1], F32, tag="totb")
            nc.scalar.copy(totb[:], ps_bc[0:P, 0:1])
            tot_eps = wp.tile([P, 1], F32, tag="toteps")
            nc.vector.tensor_scalar(out=tot_eps[:], in0=totb[:],
                                    scalar1=float(C) * EPS, scalar2=None,
                                    op0=mybir.AluOpType.add)
            rinv = wp.tile([P, 1], F32, tag="rinv")
            nc.vector.reciprocal(rinv[:], tot_eps[:])
            rfac = wp.tile([P, 1], F32, tag="rfac")
            nc.vector.tensor_tensor(out=rfac[:], in0=totb[:], in1=rinv[:],
                                    op=mybir.AluOpType.mult)

            # smoothed & reciprocal (code-tile layout)
            sm_cd = wp.tile([P, 16], F32, tag="smcd")
            nc.vector.tensor_scalar(out=sm_cd[:], in0=ncs_cd[:],
                                    scalar1=EPS, scalar2=rfac[:],
                                    op0=mybir.AluOpType.add,
                                    op1=mybir.AluOpType.mult)
            rsm_cd = wp.tile([P, 16], F32, tag="rsmcd")
            nc.vector.reciprocal(rsm_cd[:], sm_cd[:])

            # per code-tile EMA
            for i in range(16):
                es_cd = wp.tile([P, D], F32, tag="escd", bufs=2)
                for k in range(2):
                    ps_r = pp.tile([P, C], F32, tag="ps")
                    nc.tensor.transpose(
                        ps_r[0:P, 0:P],
                        redT[:, k, i * P:(i + 1) * P], ident[:])
                    nc.scalar.copy(es_cd[:, k * P:(k + 1) * P],
                                   ps_r[0:P, 0:P])
                eavg_i = wp.tile([P, D], F32, tag="eavgi", bufs=2)
                nc.sync.dma_start(eavg_i[:], eavg_in[i * P:(i + 1) * P, :])
                nea_i = wp.tile([P, D], F32, tag="neai", bufs=2)
                nc.vector.tensor_scalar(out=nea_i[:], in0=eavg_i[:],
                                        scalar1=DECAY, scalar2=None,
                                        op0=mybir.AluOpType.mult)
                tmp_i = wp.tile([P, D], F32, tag="tmpi", bufs=2)
                nc.vector.tensor_scalar(out=tmp_i[:], in0=es_cd[:],
                                        scalar1=1.0 - DECAY, scalar2=None,
                                        op0=mybir.AluOpType.mult)
                nc.vector.tensor_tensor(out=nea_i[:], in0=nea_i[:],
                                        in1=tmp_i[:], op=mybir.AluOpType.add)
                nc.sync.dma_start(nea_out[i * P:(i + 1) * P, :], nea_i[:])
                ne_i = wp.tile([P, D], F32, tag="nei", bufs=2)
                nc.vector.tensor_scalar(out=ne_i[:], in0=nea_i[:],
                                        scalar1=rsm_cd[:, i:i + 1],
                                        scalar2=None,
                                        op0=mybir.AluOpType.mult)
                nc.sync.dma_start(ne_out[i * P:(i + 1) * P, :], ne_i[:])

    nc.compile()
    return nc


def _prep_inputs(x, embed, embed_avg, cluster_size):
    """Host-side layout/precision prep. Returns per-core in_maps."""
    x = np.asarray(x, np.float32).reshape(NTOT, D)
    embed = np.asarray(embed, np.float32).reshape(C, D)
    embed_avg = np.asarray(embed_avg, np.float32).reshape(C, D)
    cluster_size = np.asarray(cluster_size, np.float32).reshape(C)

    a = 2.0 * x                                  # exact
    ah = a.astype(BF)
    e2 = (embed.astype(np.float64) ** 2).sum(-1)
    x2 = (x.astype(np.float64) ** 2).sum(-1).astype(np.float32)

    eh = embed.astype(BF)
    el = (embed - eh.astype(np.float32)).astype(BF)
    ehT = np.ascontiguousarray(eh.astype(np.float32).T.astype(BF)
                               .reshape(2, P, C))
    elT = np.ascontiguousarray(el.astype(np.float32).T.astype(BF)
                               .reshape(2, P, C))

    e2n = np.zeros((3, C), np.float32)
    r = -e2.copy()
    for i in range(3):
        p = r.astype(np.float32).astype(BF).astype(np.float32)
        e2n[i] = p
        r = r - p
    e2n = e2n.astype(BF)

    iota = np.broadcast_to(np.arange(C, dtype=np.float32), (P, C)).copy()
    ident = np.eye(P, dtype=np.float32)
    onesb = np.ones((P, P), BF)
    onesf = np.ones((P, P), np.float32)
    cs_pm = cluster_size.reshape(P, 16).copy()
    cs_cd = np.ascontiguousarray(cluster_size.reshape(16, P).T)

    shared = {
        "ehT": ehT, "elT": elT, "e2n": e2n, "embf": embed,
        "iotaf": iota, "identf": ident, "onesb": onesb, "onesf": onesf,
        "cspm": cs_pm, "cscd": cs_cd, "eavg": embed_avg,
    }
    in_maps = []
    for c in range(NCORES):
        sl = slice(c * NSH, (c + 1) * NSH)
        ah_c = ah[sl]                            # (8192, 256) bf16
        xhT = np.ascontiguousarray(
            ah_c.astype(np.float32).T.astype(BF).reshape(2, P, NSH))
        xh = x[sl].astype(BF)
        x2_c = np.ascontiguousarray(x2[sl].reshape(NT, P).T)
        m = dict(shared)
        m.update({"xhT": xhT, "xh": xh, "x2": x2_c})
        in_maps.append(m)
    return in_maps


def _host_fixup(x, embed, cluster_size, embed_avg, outs):
    """Resolve near-tie argmax tokens exactly; patch outputs in place."""
    x64 = np.asarray(x, np.float64).reshape(NTOT, D)
    e64 = np.asarray(embed, np.float64).reshape(C, D)
    embed = np.asarray(embed, np.float32).reshape(C, D)
    e2_64 = (e64 ** 2).sum(-1)

    ind = outs["embed_ind"]
    m8 = outs["m8"]
    idx8 = outs["idx8"]
    gaps = m8[:, 0] - m8[:, 1]
    amb = np.nonzero(gaps < MARGIN)[0]
    flips = []
    for n in amb:
        cands = idx8[n].astype(np.int64)
        xv = x64[n]
        sq = e2_64[cands] - 2.0 * (e64[cands] @ xv)
        best = int(cands[np.argmin(sq)])
        if best != int(ind[n]):
            flips.append((int(n), int(ind[n]), best))
            ind[n] = best
            outs["quantize"][n] = embed[best]

    if flips:
        cs = np.asarray(cluster_size, np.float32).reshape(C)
        eavg = np.asarray(embed_avg, np.float32).reshape(C, D)
        esum = outs["esum_red"][0:2 * P, :]      # (256, 2048) d-major
        bins = outs["esum_red"][2 * P, :].copy()  # (2048,)
        esum_cd = np.ascontiguousarray(esum.T)    # (2048, 256)
        xh32 = np.asarray(x, np.float32).reshape(NTOT, D).astype(BF)\
            .astype(np.float32)
        for n, old, new in flips:
            bins[old] -= 1.0
            bins[new] += 1.0
            esum_cd[old] -= xh32[n]
            esum_cd[new] += xh32[n]
        ncs_full = outs["new_cluster_size"]
        total = np.float32(ncs_full.sum(dtype=np.float32))
        rfac = np.float32(total / (total + np.float32(C * EPS)))
        affected = sorted({cd for f in flips for cd in f[1:]})
        for cd in affected:
            ncs = np.float32(DECAY) * cs[cd] + np.float32(1 - DECAY) * bins[cd]
            ncs_full[cd] = ncs
            nea = (np.float32(DECAY) * eavg[cd]
                   + np.float32(1 - DECAY) * esum_cd[cd])
            outs["new_embed_avg"][cd] = nea
            sm = (ncs + np.float32(EPS)) * rfac
            outs["new_embed"][cd] = nea * (np.float32(1.0) / sm)
    outs["n_ambiguous"] = len(amb)
    outs["n_flips"] = len(flips)
    return outs


_last_exec_ns = None
_last_profile = None


def kernel(x, embed, embed_avg, cluster_size):
    global _last_exec_ns, _last_profile
    import os
    if "nc" not in _cached:
        _cached["nc"] = _build()
    nc = _cached["nc"]
    in_maps = _prep_inputs(x, embed, embed_avg, cluster_size)
    trace = bool(os.environ.get("VQ_TRACE"))
    res = bass_utils.run_bass_kernel_spmd(
        nc, in_maps, core_ids=list(range(NCORES)), trace=trace)
    _last_exec_ns = res.exec_time_ns
    _last_profile = res
    rs = res.results

    dist = np.concatenate([r["dist_o"] for r in rs], 0).reshape(1, NTOT, C)
    quant = np.concatenate([r["quant_o"] for r in rs], 0)
    ind = np.concatenate([r["ind_o"].reshape(NSH) for r in rs], 0)\
        .astype(np.int32)
    m8 = np.concatenate(
        [r["m8_o"].reshape(P, NT, 8).transpose(1, 0, 2).reshape(NSH, 8)
         for r in rs], 0)
    idx8 = np.concatenate(
        [r["idx8_o"].reshape(P, NT, 8).transpose(1, 0, 2).reshape(NSH, 8)
         for r in rs], 0)
    r0 = rs[0]
    outs = {
        "quantize": quant,
        "embed_ind": ind,
        "dist": dist,
        "new_embed": r0["ne_o"].copy(),
        "new_cluster_size": r0["ncs_o"].reshape(C).copy(),
        "new_embed_avg": r0["nea_o"].copy(),
        "esum_red": r0["esr_o"],
        "m8": m8,
        "idx8": idx8,
    }
    _host_fixup(x, embed, cluster_size, embed_avg, outs)

    h, b, n, d = 1, 32, 2048, 256
    return (
        outs["quantize"].reshape(h, b, n, d),
        outs["embed_ind"].reshape(h, b, n),
        outs["dist"],
        outs["new_embed"].reshape(h, C, D),
        outs["new_cluster_size"].reshape(h, C),
        outs["new_embed_avg"].reshape(h, C, D),
    )


# revision 16
# speedup vs baseline: 127.8314x; 1.1731x over previous
"""EuclideanCodebook VQ kernel for 8 Trainium2 NeuronCores (Bass/Tile).

Strategy (data-parallel over tokens, codebook replicated):
  - host: bf16 hi-split of 2x (transposed) and embed (transposed), exact
    fp64->fp32 x2/e2 constants, sharding/layout prep.
  - device per core (8192 tokens): T = 2*x@e.T - e2 via bf16 matmuls into
    PSUM (exact products, ~0.06 abs noise from dropped lo-terms);
    dist = -sqrt(x2 - T) via ACT; top-8 max/argmax via DVE; onehot via
    is_equal(iota, idx); embed_sum/bins via onehot matmul accumulated in
    PSUM; quantize via indirect-DMA gather of fp32 embed rows; AllReduce
    of (embed_sum||bins) across the 8 cores; EMA epilogue on device.
  - host: exact resolution of near-tie argmax tokens (margin on top-2 gap)
    with row-local fixup of quantize/embed_ind/EMA outputs.
"""
import numpy as np
import ml_dtypes

import concourse.bass as bass
import concourse.bacc as bacc
import concourse.mybir as mybir
import concourse.tile as tile
from concourse import bass_utils

F32 = mybir.dt.float32
BF16 = mybir.dt.bfloat16
U32 = mybir.dt.uint32
I32 = mybir.dt.int32
BF = ml_dtypes.bfloat16

NCORES = 8
P = 128
C = 2048          # codebook size
D = 256           # feature dim
NTOT = 65536      # total tokens (32*2048)
NSH = NTOT // NCORES   # 8192 tokens per core
NT = NSH // P          # 64 tiles per core
G = 16                 # tiles per esum group
NGRP = NT // G         # 4 groups
DECAY = 0.8
EPS = 1e-5
MARGIN = 0.02     # host fixup margin on noisy top-2 dist gap
USE_COLLECTIVE = True

_cached = {}


def _build(ablate=()):
    ab = set(ablate)
    nc = bacc.Bacc("TRN2", target_bir_lowering=False, debug=False,
                   num_devices=NCORES)

    def din(name, shape, dt):
        return nc.dram_tensor(name, shape, dt, kind="ExternalInput").ap()

    def dout(name, shape, dt):
        return nc.dram_tensor(name, shape, dt, kind="ExternalOutput").ap()

    # per-core inputs
    xhT_in = din("xhT", [2, P, NSH], BF16)     # (2x) hi, d-major transposed
    xh_in = din("xh", [NSH, D], BF16)          # x hi, natural (for esum)
    x2_in = din("x2", [P, NT], F32)            # x2[p,t] = |x_{t*128+p}|^2
    ehT_in = din("ehT", [2, P, C], BF16)       # embed hi, transposed
    elT_in = din("elT", [2, P, C], BF16)       # embed lo, transposed
    e2n_in = din("e2n", [3, C], BF16)          # -e2 in 3 bf16 pieces
    emb_in = din("embf", [C, D], F32)          # exact embed (gather source)
    iota_in = din("iotaf", [P, C], F32)
    ident_in = din("identf", [P, P], F32)
    onesb_in = din("onesb", [P, P], BF16)
    onesf_in = din("onesf", [P, P], F32)
    cs_pm_in = din("cspm", [P, 16], F32)       # cluster_size, c = 16p+j
    cs_cd_in = din("cscd", [P, 16], F32)       # cluster_size, c = 128j+p
    eavg_in = din("eavg", [C, D], F32)

    # per-core outputs
    dist_out = dout("dist_o", [NSH, C], F32)
    quant_out = dout("quant_o", [NSH, D], F32)
    ind_out = dout("ind_o", [NT, P], I32)      # ind[t, p] = token t*128+p
    m8_out = dout("m8_o", [P, NT * 8], F32)
    idx8_out = dout("idx8_o", [P, NT * 8], U32)
    esumred_out = dout("esr_o", [257, C], F32)  # reduced (esumT || bins)
    ncs_out = dout("ncs_o", [P, 16], F32)       # new_cluster_size, c=16p+j
    nea_out = dout("nea_o", [C, D], F32)        # new_embed_avg
    ne_out = dout("ne_o", [C, D], F32)          # new_embed

    with tile.TileContext(nc) as tc:
        with (
            tc.tile_pool(name="const", bufs=1) as cp,
            tc.tile_pool(name="work", bufs=1) as wp,
            tc.tile_pool(name="ps", bufs=2, space="PSUM") as pp,
            tc.tile_pool(name="dram", bufs=1, space="DRAM") as dp,
        ):
            # ---------------- constants ----------------
            ehT = cp.tile([P, 2, C], BF16, tag="ehT")
            for k in range(2):
                nc.sync.dma_start(ehT[:, k, :], ehT_in[k])
            e2n = cp.tile([P, C], BF16, tag="e2n")
            nc.sync.dma_start(e2n[0:3, :], e2n_in[:])
            iota_t = cp.tile([P, C], F32, tag="iota")
            nc.sync.dma_start(iota_t[:], iota_in[:])
            ident = cp.tile([P, P], F32, tag="ident")
            nc.sync.dma_start(ident[:], ident_in[:])
            onesb = cp.tile([P, P], BF16, tag="onesb")
            nc.sync.dma_start(onesb[:], onesb_in[:])
            onesf = cp.tile([P, P], F32, tag="onesf")
            nc.sync.dma_start(onesf[:], onesf_in[:])
            x2sb = cp.tile([P, NT], F32, tag="x2sb")
            nc.sync.dma_start(x2sb[:], x2_in[:])

            m8st = cp.tile([P, NT * 8], F32, tag="m8st")
            idx8st = cp.tile([P, NT * 8], U32, tag="idx8st")
            idxf = cp.tile([P, NT], F32, tag="idxf")
            esacc = cp.tile([P, 3, C], F32, tag="esacc")
            nc.vector.memset(esacc[:], 0.0)

            # ---------------- main loop ----------------
            for g in range(NGRP):
                tiles = range(g * G, (g + 1) * G)
                o_tiles = {}
                xh_tiles = {}
                for t in tiles:
                    # loads
                    aht = wp.tile([P, 2, P], BF16, tag="aht", bufs=6)
                    for k in range(2):
                        nc.sync.dma_start(
                            aht[:, k, :], xhT_in[k][:, t * P:(t + 1) * P])
                    xh_t = wp.tile([P, D], BF16, tag="xh", bufs=G + 1)
                    nc.sync.dma_start(xh_t[:], xh_in[t * P:(t + 1) * P, :])
                    xh_tiles[t] = xh_t

                    # T = 2 x e^T - e2  (bf16 hi products only),
                    # computed in two 2-bank psum halves to decouple the
                    # esum phases from the main-loop psum rotation
                    d_t = wp.tile([P, C], F32, tag="dist", bufs=4)
                    for h in range(2):
                        ps_h = pp.tile([P, C // 2], F32, tag="psm", bufs=3)
                        for k in range(2):
                            for q in range(2):
                                nc.tensor.matmul(
                                    ps_h[:, q * 512:(q + 1) * 512],
                                    lhsT=aht[:, k, :],
                                    rhs=ehT[:, k, h * 1024 + q * 512:
                                            h * 1024 + (q + 1) * 512],
                                    start=(k == 0), stop=False)
                        for q in range(2):
                            nc.tensor.matmul(
                                ps_h[:, q * 512:(q + 1) * 512],
                                lhsT=onesb[0:3, :],
                                rhs=e2n[0:3, h * 1024 + q * 512:
                                        h * 1024 + (q + 1) * 512],
                                start=False, stop=(q == 1))
                        # dist = -sqrt(x2 - T)
                        if "act" not in ab:
                            nc.scalar.activation(
                                d_t[:, h * 1024:(h + 1) * 1024], ps_h[:],
                                mybir.ActivationFunctionType.Sqrt,
                                bias=x2sb[:, t:t + 1], scale=-1.0)
                    if "act" not in ab:
                        nc.scalar.mul(d_t[:], d_t[:], -1.0)
                    if "distdma" not in ab and "act" not in ab:
                        nc.sync.dma_start(
                            dist_out[t * P:(t + 1) * P, :], d_t[:])

                    # top-8 + indices from the (negated) dist tile in SBUF
                    if "argmax" not in ab:
                        m8 = m8st[:, t * 8:(t + 1) * 8]
                        nc.vector.max(m8, d_t[:])
                        i8 = idx8st[:, t * 8:(t + 1) * 8]
                        nc.vector.max_index(i8, m8, d_t[:])
                        nc.vector.tensor_copy(idxf[:, t:t + 1],
                                              idx8st[:, t * 8:t * 8 + 1])
                    # onehot
                    o_t = wp.tile([P, C], BF16, tag="oh", bufs=G + 1)
                    if "onehot" not in ab:
                        eng = nc.vector
                        eng.tensor_scalar(
                            out=o_t[:], in0=iota_t[:],
                            scalar1=idxf[:, t:t + 1], scalar2=None,
                            op0=mybir.AluOpType.is_equal)
                    o_tiles[t] = o_t

                    # quantize gather (exact fp32 embed rows)
                    if "gather" not in ab:
                        q_t = wp.tile([P, D], F32, tag="qt", bufs=3)
                        nc.gpsimd.indirect_dma_start(
                            out=q_t[:], out_offset=None, in_=emb_in[:],
                            in_offset=bass.IndirectOffsetOnAxis(
                                ap=idx8st[:, t * 8:t * 8 + 1], axis=0))
                        nc.sync.dma_start(
                            quant_out[t * P:(t + 1) * P, :], q_t[:])

                # ---- esum phase for this group ----
                for cch in (() if "esum" in ab else range(3)):
                    mrows = P if cch < 2 else 1
                    for h in range(2):
                        ps_e = pp.tile([P, C // 2], F32, tag="pse", bufs=1)
                        for i, t in enumerate(tiles):
                            if cch < 2:
                                lhsT = xh_tiles[t][:, cch * P:(cch + 1) * P]
                            else:
                                lhsT = onesb[:, 0:1]
                            for q in range(2):
                                nc.tensor.matmul(
                                    ps_e[0:mrows, q * 512:(q + 1) * 512],
                                    lhsT=lhsT,
                                    rhs=o_tiles[t][:, h * 1024 + q * 512:
                                                   h * 1024 + (q + 1) * 512],
                                    start=(i == 0), stop=(i == G - 1))
                        sl = slice(h * 1024, (h + 1) * 1024)
                        nc.vector.tensor_tensor(
                            out=esacc[0:mrows, cch, sl],
                            in0=esacc[0:mrows, cch, sl],
                            in1=ps_e[0:mrows, :], op=mybir.AluOpType.add)

            # ---------------- embed_ind output ----------------
            ps_i = pp.tile([P, C // 2], F32, tag="psm", bufs=3)
            nc.tensor.transpose(ps_i[0:NT, 0:P], idxf[:, 0:NT], ident[:])
            ind_sb = wp.tile([NT, P], I32, tag="indsb")
            nc.vector.tensor_copy(ind_sb[:], ps_i[0:NT, 0:P])
            nc.sync.dma_start(ind_out[:], ind_sb[:])
            nc.sync.dma_start(m8_out[:], m8st[:])
            nc.sync.dma_start(idx8_out[:], idx8st[:])

            # ---------------- all-reduce esum ----------------
            part = dp.tile([257, C], F32)
            nc.sync.dma_start(part[0:P, :], esacc[:, 0, :])
            nc.sync.dma_start(part[P:2 * P, :], esacc[:, 1, :])
            nc.sync.dma_start(part[2 * P:257, :], esacc[0:1, 2, :])
            if USE_COLLECTIVE:
                red = dp.tile([257, C], F32, addr_space="Shared")
                nc.gpsimd.collective_compute(
                    "AllReduce", mybir.AluOpType.add,
                    replica_groups=[list(range(NCORES))],
                    ins=[part.opt()], outs=[red.opt()])
            else:
                red = part
            nc.sync.dma_start(esumred_out[:], red[:])

            # load reduced back
            redT = wp.tile([P, 2, C], F32, tag="redT")
            nc.sync.dma_start(redT[:, 0, :], red[0:P, :])
            nc.sync.dma_start(redT[:, 1, :], red[P:2 * P, :])
            binsrow = wp.tile([P, C], F32, tag="binsrow")
            nc.sync.dma_start(binsrow[0:1, :], red[2 * P:257, :])
            binspm = wp.tile([P, 16], F32, tag="binspm")
            nc.sync.dma_start(
                binspm[:],
                red[2 * P:257, :].rearrange("one (p j) -> (one p) j", p=P))

            cs_pm = wp.tile([P, 16], F32, tag="cspm")
            nc.sync.dma_start(cs_pm[:], cs_pm_in[:])
            cs_cd = wp.tile([P, 16], F32, tag="cscd")
            nc.sync.dma_start(cs_cd[:], cs_cd_in[:])

            # bins in code-tile layout via 16 mini transposes
            binscd = wp.tile([P, 16], F32, tag="binscd")
            for i in range(16):
                ps_b = pp.tile([P, C // 2], F32, tag="psm", bufs=3)
                nc.tensor.transpose(
                    ps_b[0:P, 0:1], binsrow[0:1, i * P:(i + 1) * P],
                    ident[0:1, 0:1])
                nc.scalar.copy(binscd[:, i:i + 1], ps_b[0:P, 0:1])

            # ncs (both layouts)
            ncs_pm = wp.tile([P, 16], F32, tag="ncspm")
            nc.vector.tensor_scalar(out=ncs_pm[:], in0=cs_pm[:],
                                    scalar1=DECAY, scalar2=None,
                                    op0=mybir.AluOpType.mult)
            tmp_pm = wp.tile([P, 16], F32, tag="tmppm")
            nc.vector.tensor_scalar(out=tmp_pm[:], in0=binspm[:],
                                    scalar1=1.0 - DECAY, scalar2=None,
                                    op0=mybir.AluOpType.mult)
            nc.vector.tensor_tensor(out=ncs_pm[:], in0=ncs_pm[:],
                                    in1=tmp_pm[:], op=mybir.AluOpType.add)
            nc.sync.dma_start(ncs_out[:], ncs_pm[:])

            ncs_cd = wp.tile([P, 16], F32, tag="ncscd")
            nc.vector.tensor_scalar(out=ncs_cd[:], in0=cs_cd[:],
                                    scalar1=DECAY, scalar2=None,
                                    op0=mybir.AluOpType.mult)
            tmp_cd = wp.tile([P, 16], F32, tag="tmpcd")
            nc.vector.tensor_scalar(out=tmp_cd[:], in0=binscd[:],
                                    scalar1=1.0 - DECAY, scalar2=None,
                                    op0=mybir.AluOpType.mult)
            nc.vector.tensor_tensor(out=ncs_cd[:], in0=ncs_cd[:],
                                    in1=tmp_cd[:], op=mybir.AluOpType.add)

            # total = sum(ncs); r = total / (total + C*EPS)
            rowsum = wp.tile([P, 1], F32, tag="rowsum")
            nc.vector.tensor_reduce(rowsum[:], ncs_pm[:],
                                    axis=mybir.AxisListType.X,
                                    op=mybir.AluOpType.add)
            ps_s = pp.tile([P, C // 2], F32, tag="psm", bufs=3)
            nc.tensor.matmul(ps_s[0:1, 0:1], lhsT=onesf[:, 0:1],
                             rhs=rowsum[:], start=True, stop=True)
            t11 = wp.tile([P, 1], F32, tag="t11")
            nc.scalar.copy(t11[0:1, :], ps_s[0:1, 0:1])
            ps_bc = pp.tile([P, C // 2], F32, tag="psm", bufs=3)
            nc.tensor.matmul(ps_bc[0:P, 0:1], lhsT=onesf[0:1, :],
                             rhs=t11[0:1, 0:1], start=True, stop=True)
            totb = wp.tile([P, 1], F32, tag="totb")
            nc.scalar.copy(totb[:], ps_bc[0:P, 0:1])
            tot_eps = wp.tile([P, 1], F32, tag="toteps")
            nc.vector.tensor_scalar(out=tot_eps[:], in0=totb[:],
                                    scalar1=float(C) * EPS, scalar2=None,
                                    op0=mybir.AluOpType.add)
            rinv = wp.tile([P, 1], F32, tag="rinv")
            nc.vector.reciprocal(rinv[:], tot_eps[:])
            rfac = wp.tile([P, 1], F32, tag="rfac")
            nc.vector.tensor_tensor(out=rfac[:], in0=totb[:], in1=rinv[:],
                                    op=mybir.AluOpType.mult)

            # smoothed & reciprocal (code-tile layout)
            sm_cd = wp.tile([P, 16], F32, tag="smcd")
            nc.vector.tensor_scalar(out=sm_cd[:], in0=ncs_cd[:],
                                    scalar1=EPS, scalar2=rfac[:],
                                    op0=mybir.AluOpType.add,
                                    op1=mybir.AluOpType.mult)
            rsm_cd = wp.tile([P, 16], F32, tag="rsmcd")
            nc.vector.reciprocal(rsm_cd[:], sm_cd[:])

            # per code-tile EMA
            for i in range(16):
                es_cd = wp.tile([P, D], F32, tag="escd", bufs=2)
                for k in range(2):
                    ps_r = pp.tile([P, C // 2], F32, tag="psm", bufs=3)
                    nc.tensor.transpose(
                        ps_r[0:P, 0:P],
                        redT[:, k, i * P:(i + 1) * P], ident[:])
                    nc.scalar.copy(es_cd[:, k * P:(k + 1) * P],
                                   ps_r[0:P, 0:P])
                eavg_i = wp.tile([P, D], F32, tag="eavgi", bufs=2)
                nc.sync.dma_start(eavg_i[:], eavg_in[i * P:(i + 1) * P, :])
                nea_i = wp.tile([P, D], F32, tag="neai", bufs=2)
                nc.vector.tensor_scalar(out=nea_i[:], in0=eavg_i[:],
                                        scalar1=DECAY, scalar2=None,
                                        op0=mybir.AluOpType.mult)
                tmp_i = wp.tile([P, D], F32, tag="tmpi", bufs=2)
                nc.vector.tensor_scalar(out=tmp_i[:], in0=es_cd[:],
                                        scalar1=1.0 - DECAY, scalar2=None,
                                        op0=mybir.AluOpType.mult)
                nc.vector.tensor_tensor(out=nea_i[:], in0=nea_i[:],
                                        in1=tmp_i[:], op=mybir.AluOpType.add)
                nc.sync.dma_start(nea_out[i * P:(i + 1) * P, :], nea_i[:])
                ne_i = wp.tile([P, D], F32, tag="nei", bufs=2)
                nc.vector.tensor_scalar(out=ne_i[:], in0=nea_i[:],
                                        scalar1=rsm_cd[:, i:i + 1],
                                        scalar2=None,
                                        op0=mybir.AluOpType.mult)
                nc.sync.dma_start(ne_out[i * P:(i + 1) * P, :], ne_i[:])

    nc.compile()
    return nc


def _prep_inputs(x, embed, embed_avg, cluster_size):
    """Host-side layout/precision prep. Returns per-core in_maps."""
    x = np.asarray(x, np.float32).reshape(NTOT, D)
    embed = np.asarray(embed, np.float32).reshape(C, D)
    embed_avg = np.asarray(embed_avg, np.float32).reshape(C, D)
    cluster_size = np.asarray(cluster_size, np.float32).reshape(C)

    a = 2.0 * x                                  # exact
    ah = a.astype(BF)
    e2 = (embed.astype(np.float64) ** 2).sum(-1)
    x2 = (x.astype(np.float64) ** 2).sum(-1).astype(np.float32)

    eh = embed.astype(BF)
    el = (embed - eh.astype(np.float32)).astype(BF)
    ehT = np.ascontiguousarray(eh.astype(np.float32).T.astype(BF)
                               .reshape(2, P, C))
    elT = np.ascontiguousarray(el.astype(np.float32).T.astype(BF)
                               .reshape(2, P, C))

    e2n = np.zeros((3, C), np.float32)
    r = -e2.copy()
    for i in range(3):
        p = r.astype(np.float32).astype(BF).astype(np.float32)
        e2n[i] = p
        r = r - p
    e2n = e2n.astype(BF)

    iota = np.broadcast_to(np.arange(C, dtype=np.float32), (P, C)).copy()
    ident = np.eye(P, dtype=np.float32)
    onesb = np.ones((P, P), BF)
    onesf = np.ones((P, P), np.float32)
    cs_pm = cluster_size.reshape(P, 16).copy()
    cs_cd = np.ascontiguousarray(cluster_size.reshape(16, P).T)

    shared = {
        "ehT": ehT, "elT": elT, "e2n": e2n, "embf": embed,
        "iotaf": iota, "identf": ident, "onesb": onesb, "onesf": onesf,
        "cspm": cs_pm, "cscd": cs_cd, "eavg": embed_avg,
    }
    in_maps = []
    for c in range(NCORES):
        sl = slice(c * NSH, (c + 1) * NSH)
        ah_c = ah[sl]                            # (8192, 256) bf16
        xhT = np.ascontiguousarray(
            ah_c.astype(np.float32).T.astype(BF).reshape(2, P, NSH))
        xh = x[sl].astype(BF)
        x2_c = np.ascontiguousarray(x2[sl].reshape(NT, P).T)
        m = dict(shared)
        m.update({"xhT": xhT, "xh": xh, "x2": x2_c})
        in_maps.append(m)
    return in_maps


def _host_fixup(x, embed, cluster_size, embed_avg, outs):
    """Resolve near-tie argmax tokens exactly; patch outputs in place."""
    x64 = np.asarray(x, np.float64).reshape(NTOT, D)
    e64 = np.asarray(embed, np.float64).reshape(C, D)
    embed = np.asarray(embed, np.float32).reshape(C, D)
    e2_64 = (e64 ** 2).sum(-1)

    ind = outs["embed_ind"]
    m8 = outs["m8"]
    idx8 = outs["idx8"]
    gaps = m8[:, 0] - m8[:, 1]
    amb = np.nonzero(gaps < MARGIN)[0]
    flips = []
    for n in amb:
        cands = idx8[n].astype(np.int64)
        xv = x64[n]
        sq = e2_64[cands] - 2.0 * (e64[cands] @ xv)
        best = int(cands[np.argmin(sq)])
        if best != int(ind[n]):
            flips.append((int(n), int(ind[n]), best))
            ind[n] = best
            outs["quantize"][n] = embed[best]

    if flips:
        cs = np.asarray(cluster_size, np.float32).reshape(C)
        eavg = np.asarray(embed_avg, np.float32).reshape(C, D)
        esum = outs["esum_red"][0:2 * P, :]      # (256, 2048) d-major
        bins = outs["esum_red"][2 * P, :].copy()  # (2048,)
        esum_cd = np.ascontiguousarray(esum.T)    # (2048, 256)
        xh32 = np.asarray(x, np.float32).reshape(NTOT, D).astype(BF)\
            .astype(np.float32)
        for n, old, new in flips:
            bins[old] -= 1.0
            bins[new] += 1.0
            esum_cd[old] -= xh32[n]
            esum_cd[new] += xh32[n]
        ncs_full = outs["new_cluster_size"]
        total = np.float32(ncs_full.sum(dtype=np.float32))
        rfac = np.float32(total / (total + np.float32(C * EPS)))
        affected = sorted({cd for f in flips for cd in f[1:]})
        for cd in affected:
            ncs = np.float32(DECAY) * cs[cd] + np.float32(1 - DECAY) * bins[cd]
            ncs_full[cd] = ncs
            nea = (np.float32(DECAY) * eavg[cd]
                   + np.float32(1 - DECAY) * esum_cd[cd])
            outs["new_embed_avg"][cd] = nea
            sm = (ncs + np.float32(EPS)) * rfac
            outs["new_embed"][cd] = nea * (np.float32(1.0) / sm)
    outs["n_ambiguous"] = len(amb)
    outs["n_flips"] = len(flips)
    return outs


_last_exec_ns = None
_last_profile = None


def kernel(x, embed, embed_avg, cluster_size):
    global _last_exec_ns, _last_profile
    import os
    if "nc" not in _cached:
        _cached["nc"] = _build()
    nc = _cached["nc"]
    in_maps = _prep_inputs(x, embed, embed_avg, cluster_size)
    trace = bool(os.environ.get("VQ_TRACE"))
    res = bass_utils.run_bass_kernel_spmd(
        nc, in_maps, core_ids=list(range(NCORES)), trace=trace)
    _last_exec_ns = res.exec_time_ns
    _last_profile = res
    rs = res.results

    dist = np.concatenate([r["dist_o"] for r in rs], 0).reshape(1, NTOT, C)
    quant = np.concatenate([r["quant_o"] for r in rs], 0)
    ind = np.concatenate([r["ind_o"].reshape(NSH) for r in rs], 0)\
        .astype(np.int32)
    m8 = np.concatenate(
        [r["m8_o"].reshape(P, NT, 8).transpose(1, 0, 2).reshape(NSH, 8)
         for r in rs], 0)
    idx8 = np.concatenate(
        [r["idx8_o"].reshape(P, NT, 8).transpose(1, 0, 2).reshape(NSH, 8)
         for r in rs], 0)
    r0 = rs[0]
    outs = {
        "quantize": quant,
        "embed_ind": ind,
        "dist": dist,
        "new_embed": r0["ne_o"].copy(),
        "new_cluster_size": r0["ncs_o"].reshape(C).copy(),
        "new_embed_avg": r0["nea_o"].copy(),
        "esum_red": r0["esr_o"],
        "m8": m8,
        "idx8": idx8,
    }
    _host_fixup(x, embed, cluster_size, embed_avg, outs)

    h, b, n, d = 1, 32, 2048, 256
    return (
        outs["quantize"].reshape(h, b, n, d),
        outs["embed_ind"].reshape(h, b, n),
        outs["dist"],
        outs["new_embed"].reshape(h, C, D),
        outs["new_cluster_size"].reshape(h, C),
        outs["new_embed_avg"].reshape(h, C, D),
    )


# revision 20
# speedup vs baseline: 161.2101x; 1.2611x over previous
"""EuclideanCodebook VQ kernel for 8 Trainium2 NeuronCores (Bass/Tile).

Strategy (data-parallel over tokens, codebook replicated):
  - host: bf16 hi-split of 2x (transposed) and embed (transposed), exact
    fp64->fp32 x2/e2 constants, sharding/layout prep.
  - device per core (8192 tokens): T = 2*x@e.T - e2 via bf16 matmuls into
    PSUM (exact products, ~0.06 abs noise from dropped lo-terms);
    dist = -sqrt(x2 - T) via ACT; top-8 max/argmax via DVE; onehot via
    is_equal(iota, idx); embed_sum/bins via onehot matmul accumulated in
    PSUM; quantize via indirect-DMA gather of fp32 embed rows; AllReduce
    of (embed_sum||bins) across the 8 cores; EMA epilogue on device.
  - host: exact resolution of near-tie argmax tokens (margin on top-2 gap)
    with row-local fixup of quantize/embed_ind/EMA outputs.
"""
import numpy as np
import ml_dtypes

import concourse.bass as bass
import concourse.bacc as bacc
import concourse.mybir as mybir
import concourse.tile as tile
from concourse import bass_utils

F32 = mybir.dt.float32
BF16 = mybir.dt.bfloat16
U32 = mybir.dt.uint32
I32 = mybir.dt.int32
BF = ml_dtypes.bfloat16

NCORES = 8
P = 128
C = 2048          # codebook size
D = 256           # feature dim
NTOT = 65536      # total tokens (32*2048)
NSH = NTOT // NCORES   # 8192 tokens per core
NT = NSH // P          # 64 tiles per core
G = 16                 # tiles per esum group
NGRP = NT // G         # 4 groups
DECAY = 0.8
EPS = 1e-5
MARGIN = 0.02     # host fixup margin on noisy top-2 dist gap
USE_COLLECTIVE = True

_cached = {}


def _build(ablate=()):
    ab = set(ablate)
    nc = bacc.Bacc("TRN2", target_bir_lowering=False, debug=False,
                   num_devices=NCORES)

    def din(name, shape, dt):
        return nc.dram_tensor(name, shape, dt, kind="ExternalInput").ap()

    def dout(name, shape, dt):
        return nc.dram_tensor(name, shape, dt, kind="ExternalOutput").ap()

    # per-core inputs
    xhT_in = din("xhT", [2, P, NSH], BF16)     # (2x) hi, d-major transposed
    xh_in = din("xh", [NSH, D], BF16)          # x hi, natural (for esum)
    x2_in = din("x2", [P, NT], F32)            # x2[p,t] = |x_{t*128+p}|^2
    ehT_in = din("ehT", [2, P, C], BF16)       # embed hi, transposed
    elT_in = din("elT", [2, P, C], BF16)       # embed lo, transposed
    e2n_in = din("e2n", [3, C], BF16)          # -e2 in 3 bf16 pieces
    emb_in = din("embf", [C, D], F32)          # exact embed (gather source)
    iota_in = din("iotaf", [P, C], F32)
    ident_in = din("identf", [P, P], F32)
    onesb_in = din("onesb", [P, P], BF16)
    onesf_in = din("onesf", [P, P], F32)
    cs_sh_in = din("cssh", [P, 2], F32)        # this core's cs, col i = codes
    rfac_in = din("rfac", [P, 1], F32)         # total/(total+C*EPS), replicated
    eavg_in = din("eavg", [2, P, D], F32)      # this core's embed_avg rows

    # per-core outputs
    dist_out = dout("dist_o", [NSH, C], F32)
    quant_out = dout("quant_o", [NSH, D], F32)
    ind_out = dout("ind_o", [NT, P], I32)      # ind[t, p] = token t*128+p
    m8_out = dout("m8_o", [P, NT * 8], F32)
    idx8_out = dout("idx8_o", [P, NT * 8], U32)
    esumred_out = dout("esr_o", [2 * P, 257], F32)  # this core's reduced slice
    ncs_out = dout("ncs_o", [P, 2], F32)        # new_cluster_size slice
    nea_out = dout("nea_o", [2, P, D], F32)     # new_embed_avg slice
    ne_out = dout("ne_o", [2, P, D], F32)       # new_embed slice

    with tile.TileContext(nc) as tc:
        with (
            tc.tile_pool(name="const", bufs=1) as cp,
            tc.tile_pool(name="work", bufs=1) as wp,
            tc.tile_pool(name="ps", bufs=2, space="PSUM") as pp,
            tc.tile_pool(name="dram", bufs=1, space="DRAM") as dp,
        ):
            # ---------------- constants ----------------
            ehT = cp.tile([P, 2, C], BF16, tag="ehT")
            for k in range(2):
                nc.sync.dma_start(ehT[:, k, :], ehT_in[k])
            e2n = cp.tile([P, C], BF16, tag="e2n")
            nc.sync.dma_start(e2n[0:3, :], e2n_in[:])
            iota_t = cp.tile([P, C], F32, tag="iota")
            nc.sync.dma_start(iota_t[:], iota_in[:])
            ident = cp.tile([P, P], F32, tag="ident")
            nc.sync.dma_start(ident[:], ident_in[:])
            onesb = cp.tile([P, P], BF16, tag="onesb")
            nc.sync.dma_start(onesb[:], onesb_in[:])
            onesf = cp.tile([P, P], F32, tag="onesf")
            nc.sync.dma_start(onesf[:], onesf_in[:])
            x2sb = cp.tile([P, NT], F32, tag="x2sb")
            nc.sync.dma_start(x2sb[:], x2_in[:])

            m8st = cp.tile([P, NT * 8], F32, tag="m8st")
            idx8st = cp.tile([P, NT * 8], U32, tag="idx8st")
            idxf = cp.tile([P, NT], F32, tag="idxf")
            esacc = cp.tile([P, 3, C], F32, tag="esacc")
            ncs_out_sb = cp.tile([P, 2], F32, tag="ncsst")
            nc.vector.memset(esacc[:], 0.0)

            # ---------------- main loop ----------------
            for g in range(NGRP):
                tiles = range(g * G, (g + 1) * G)
                o_tiles = {}
                xh_tiles = {}
                for t in tiles:
                    # loads
                    aht = wp.tile([P, 2, P], BF16, tag="aht", bufs=6)
                    for k in range(2):
                        nc.sync.dma_start(
                            aht[:, k, :], xhT_in[k][:, t * P:(t + 1) * P])
                    xh_t = wp.tile([P, D], BF16, tag="xh", bufs=G + 1)
                    nc.sync.dma_start(xh_t[:], xh_in[t * P:(t + 1) * P, :])
                    xh_tiles[t] = xh_t

                    # T = 2 x e^T - e2  (bf16 hi products only),
                    # computed in two 2-bank psum halves to decouple the
                    # esum phases from the main-loop psum rotation
                    d_t = wp.tile([P, C], F32, tag="dist", bufs=4)
                    for h in range(2):
                        ps_h = pp.tile([P, C // 2], F32, tag="psm", bufs=3)
                        for k in range(2):
                            for q in range(2):
                                nc.tensor.matmul(
                                    ps_h[:, q * 512:(q + 1) * 512],
                                    lhsT=aht[:, k, :],
                                    rhs=ehT[:, k, h * 1024 + q * 512:
                                            h * 1024 + (q + 1) * 512],
                                    start=(k == 0), stop=False)
                        for q in range(2):
                            nc.tensor.matmul(
                                ps_h[:, q * 512:(q + 1) * 512],
                                lhsT=onesb[0:3, :],
                                rhs=e2n[0:3, h * 1024 + q * 512:
                                        h * 1024 + (q + 1) * 512],
                                start=False, stop=(q == 1))
                        # dist = -sqrt(x2 - T)
                        if "act" not in ab:
                            nc.scalar.activation(
                                d_t[:, h * 1024:(h + 1) * 1024], ps_h[:],
                                mybir.ActivationFunctionType.Sqrt,
                                bias=x2sb[:, t:t + 1], scale=-1.0)
                    if "act" not in ab:
                        nc.scalar.mul(d_t[:], d_t[:], -1.0)
                    if "distdma" not in ab and "act" not in ab:
                        nc.sync.dma_start(
                            dist_out[t * P:(t + 1) * P, :], d_t[:])

                    # top-8 + indices from the (negated) dist tile in SBUF
                    if "argmax" not in ab:
                        m8 = m8st[:, t * 8:(t + 1) * 8]
                        nc.vector.max(m8, d_t[:])
                        i8 = idx8st[:, t * 8:(t + 1) * 8]
                        nc.vector.max_index(i8, m8, d_t[:])
                        nc.vector.tensor_copy(idxf[:, t:t + 1],
                                              idx8st[:, t * 8:t * 8 + 1])
                    # onehot
                    o_t = wp.tile([P, C], BF16, tag="oh", bufs=G + 1)
                    if "onehot" not in ab:
                        eng = nc.vector
                        eng.tensor_scalar(
                            out=o_t[:], in0=iota_t[:],
                            scalar1=idxf[:, t:t + 1], scalar2=None,
                            op0=mybir.AluOpType.is_equal)
                    o_tiles[t] = o_t

                    # quantize gather (exact fp32 embed rows)
                    if "gather" not in ab:
                        q_t = wp.tile([P, D], F32, tag="qt", bufs=4)
                        nc.gpsimd.indirect_dma_start(
                            out=q_t[:], out_offset=None, in_=emb_in[:],
                            in_offset=bass.IndirectOffsetOnAxis(
                                ap=idx8st[:, t * 8:t * 8 + 1], axis=0))
                        nc.sync.dma_start(
                            quant_out[t * P:(t + 1) * P, :], q_t[:])

                # ---- esum phase for this group ----
                for cch in (() if "esum" in ab else range(3)):
                    mrows = P if cch < 2 else 1
                    for h in range(2):
                        ps_e = pp.tile([P, C // 2], F32, tag="pse", bufs=1)
                        for i, t in enumerate(tiles):
                            if cch < 2:
                                lhsT = xh_tiles[t][:, cch * P:(cch + 1) * P]
                            else:
                                lhsT = onesb[:, 0:1]
                            for q in range(2):
                                nc.tensor.matmul(
                                    ps_e[0:mrows, q * 512:(q + 1) * 512],
                                    lhsT=lhsT,
                                    rhs=o_tiles[t][:, h * 1024 + q * 512:
                                                   h * 1024 + (q + 1) * 512],
                                    start=(i == 0), stop=(i == G - 1))
                        sl = slice(h * 1024, (h + 1) * 1024)
                        nc.vector.tensor_tensor(
                            out=esacc[0:mrows, cch, sl],
                            in0=esacc[0:mrows, cch, sl],
                            in1=ps_e[0:mrows, :], op=mybir.AluOpType.add)

            # ---------------- embed_ind output ----------------
            ps_i = pp.tile([P, C // 2], F32, tag="psm", bufs=3)
            nc.tensor.transpose(ps_i[0:NT, 0:P], idxf[:, 0:NT], ident[:])
            ind_sb = wp.tile([NT, P], I32, tag="indsb")
            nc.vector.tensor_copy(ind_sb[:], ps_i[0:NT, 0:P])
            nc.sync.dma_start(ind_out[:], ind_sb[:])
            nc.sync.dma_start(m8_out[:], m8st[:])
            nc.sync.dma_start(idx8_out[:], idx8st[:])

            # ---- transpose local esum to code-major (2048, 257) ----
            part = dp.tile([C, 257], F32)
            for i in range(16):
                pcd = wp.tile([P, 257], F32, tag="qt", bufs=4)
                for k in range(2):
                    ps_r = pp.tile([P, C // 2], F32, tag="psm", bufs=3)
                    nc.tensor.transpose(
                        ps_r[0:P, 0:P],
                        esacc[:, k, i * P:(i + 1) * P], ident[:])
                    nc.scalar.copy(pcd[:, k * P:(k + 1) * P], ps_r[0:P, 0:P])
                ps_b = pp.tile([P, C // 2], F32, tag="psm", bufs=3)
                nc.tensor.transpose(
                    ps_b[0:P, 0:1], esacc[0:1, 2, i * P:(i + 1) * P],
                    ident[0:1, 0:1])
                nc.scalar.copy(pcd[:, 256:257], ps_b[0:P, 0:1])
                nc.sync.dma_start(part[i * P:(i + 1) * P, :], pcd[:])

            # ---- reduce-scatter: each core gets its 256 codes ----
            if USE_COLLECTIVE:
                red = dp.tile([2 * P, 257], F32)
                nc.gpsimd.collective_compute(
                    "ReduceScatter", mybir.AluOpType.add,
                    replica_groups=[list(range(NCORES))],
                    ins=[part.opt()], outs=[red.opt()])
            else:
                red = part
            nc.sync.dma_start(esumred_out[:], red[0:2 * P, :])

            # ---- EMA for this core's 2 code-tiles ----
            cs_sh = wp.tile([P, 2], F32, tag="small")
            nc.sync.dma_start(cs_sh[:], cs_sh_in[:])
            rfac = wp.tile([P, 1], F32, tag="small2")
            nc.sync.dma_start(rfac[:], rfac_in[:])
            for i in range(2):
                es_cd = wp.tile([P, D + 1], F32, tag="qt", bufs=4)
                nc.sync.dma_start(es_cd[:], red[i * P:(i + 1) * P, :])
                ncs_i = wp.tile([P, 1], F32, tag="ncsi", bufs=2)
                nc.vector.tensor_scalar(out=ncs_i[:], in0=es_cd[:, 256:257],
                                        scalar1=1.0 - DECAY, scalar2=None,
                                        op0=mybir.AluOpType.mult)
                cst = wp.tile([P, 1], F32, tag="csti", bufs=2)
                nc.vector.tensor_scalar(out=cst[:], in0=cs_sh[:, i:i + 1],
                                        scalar1=DECAY, scalar2=None,
                                        op0=mybir.AluOpType.mult)
                nc.vector.tensor_tensor(out=ncs_i[:], in0=ncs_i[:],
                                        in1=cst[:], op=mybir.AluOpType.add)
                nc.vector.tensor_copy(ncs_out_sb[:, i:i + 1], ncs_i[:])
                sm_i = wp.tile([P, 1], F32, tag="smi", bufs=2)
                nc.vector.tensor_scalar(out=sm_i[:], in0=ncs_i[:],
                                        scalar1=EPS, scalar2=rfac[:],
                                        op0=mybir.AluOpType.add,
                                        op1=mybir.AluOpType.mult)
                rsm_i = wp.tile([P, 1], F32, tag="rsmi", bufs=2)
                nc.vector.reciprocal(rsm_i[:], sm_i[:])
                eavg_i = wp.tile([P, D], F32, tag="qt", bufs=4)
                nc.sync.dma_start(eavg_i[:], eavg_in[i])
                nea_i = wp.tile([P, D], F32, tag="qt", bufs=4)
                nc.vector.tensor_scalar(out=nea_i[:], in0=eavg_i[:],
                                        scalar1=DECAY, scalar2=None,
                                        op0=mybir.AluOpType.mult)
                tmp_i = wp.tile([P, D], F32, tag="qt", bufs=4)
                nc.vector.tensor_scalar(out=tmp_i[:], in0=es_cd[:, 0:D],
                                        scalar1=1.0 - DECAY, scalar2=None,
                                        op0=mybir.AluOpType.mult)
                nc.vector.tensor_tensor(out=nea_i[:], in0=nea_i[:],
                                        in1=tmp_i[:], op=mybir.AluOpType.add)
                nc.sync.dma_start(nea_out[i], nea_i[:])
                ne_i = wp.tile([P, D], F32, tag="qt", bufs=4)
                nc.vector.tensor_scalar(out=ne_i[:], in0=nea_i[:],
                                        scalar1=rsm_i[:], scalar2=None,
                                        op0=mybir.AluOpType.mult)
                nc.sync.dma_start(ne_out[i], ne_i[:])
            nc.sync.dma_start(ncs_out[:], ncs_out_sb[:])

    nc.compile()
    return nc


def _prep_inputs(x, embed, embed_avg, cluster_size):
    """Host-side layout/precision prep. Returns per-core in_maps."""
    x = np.asarray(x, np.float32).reshape(NTOT, D)
    embed = np.asarray(embed, np.float32).reshape(C, D)
    embed_avg = np.asarray(embed_avg, np.float32).reshape(C, D)
    cluster_size = np.asarray(cluster_size, np.float32).reshape(C)

    a = 2.0 * x                                  # exact
    ah = a.astype(BF)
    e2 = (embed.astype(np.float64) ** 2).sum(-1)
    x2 = (x.astype(np.float64) ** 2).sum(-1).astype(np.float32)

    eh = embed.astype(BF)
    el = (embed - eh.astype(np.float32)).astype(BF)
    ehT = np.ascontiguousarray(eh.astype(np.float32).T.astype(BF)
                               .reshape(2, P, C))
    elT = np.ascontiguousarray(el.astype(np.float32).T.astype(BF)
                               .reshape(2, P, C))

    e2n = np.zeros((3, C), np.float32)
    r = -e2.copy()
    for i in range(3):
        p = r.astype(np.float32).astype(BF).astype(np.float32)
        e2n[i] = p
        r = r - p
    e2n = e2n.astype(BF)

    iota = np.broadcast_to(np.arange(C, dtype=np.float32), (P, C)).copy()
    ident = np.eye(P, dtype=np.float32)
    onesb = np.ones((P, P), BF)
    onesf = np.ones((P, P), np.float32)
    total = np.float32(np.float32(DECAY) * cluster_size.sum(dtype=np.float64)
                       + np.float32(1.0 - DECAY) * NTOT)
    rfac = np.float32(total / (total + np.float32(C * EPS)))
    rfac_t = np.full((P, 1), rfac, np.float32)

    shared = {
        "ehT": ehT, "elT": elT, "e2n": e2n, "embf": embed,
        "iotaf": iota, "identf": ident, "onesb": onesb, "onesf": onesf,
        "rfac": rfac_t,
    }
    in_maps = []
    for c in range(NCORES):
        sl = slice(c * NSH, (c + 1) * NSH)
        ah_c = ah[sl]                            # (8192, 256) bf16
        xhT = np.ascontiguousarray(
            ah_c.astype(np.float32).T.astype(BF).reshape(2, P, NSH))
        xh = x[sl].astype(BF)
        x2_c = np.ascontiguousarray(x2[sl].reshape(NT, P).T)
        m = dict(shared)
        csl = cluster_size[c * 2 * P:(c + 1) * 2 * P]
        m.update({
            "xhT": xhT, "xh": xh, "x2": x2_c,
            "cssh": np.ascontiguousarray(csl.reshape(2, P).T),
            "eavg": embed_avg[c * 2 * P:(c + 1) * 2 * P].reshape(2, P, D),
        })
        in_maps.append(m)
    return in_maps


def _host_fixup(x, embed, cluster_size, embed_avg, outs):
    """Resolve near-tie argmax tokens exactly; patch outputs in place."""
    x64 = np.asarray(x, np.float64).reshape(NTOT, D)
    e64 = np.asarray(embed, np.float64).reshape(C, D)
    embed = np.asarray(embed, np.float32).reshape(C, D)
    e2_64 = (e64 ** 2).sum(-1)

    ind = outs["embed_ind"]
    m8 = outs["m8"]
    idx8 = outs["idx8"]
    gaps = m8[:, 0] - m8[:, 1]
    amb = np.nonzero(gaps < MARGIN)[0]
    flips = []
    for n in amb:
        cands = idx8[n].astype(np.int64)
        xv = x64[n]
        sq = e2_64[cands] - 2.0 * (e64[cands] @ xv)
        best = int(cands[np.argmin(sq)])
        if best != int(ind[n]):
            flips.append((int(n), int(ind[n]), best))
            ind[n] = best
            outs["quantize"][n] = embed[best]

    if flips:
        cs = np.asarray(cluster_size, np.float32).reshape(C)
        eavg = np.asarray(embed_avg, np.float32).reshape(C, D)
        esum_cd = outs["esum_red"][:, 0:D].copy()   # (2048, 256)
        bins = outs["esum_red"][:, D].copy()        # (2048,)
        xh32 = np.asarray(x, np.float32).reshape(NTOT, D).astype(BF)\
            .astype(np.float32)
        for n, old, new in flips:
            bins[old] -= 1.0
            bins[new] += 1.0
            esum_cd[old] -= xh32[n]
            esum_cd[new] += xh32[n]
        ncs_full = outs["new_cluster_size"]
        total = np.float32(ncs_full.sum(dtype=np.float32))
        rfac = np.float32(total / (total + np.float32(C * EPS)))
        affected = sorted({cd for f in flips for cd in f[1:]})
        for cd in affected:
            ncs = np.float32(DECAY) * cs[cd] + np.float32(1 - DECAY) * bins[cd]
            ncs_full[cd] = ncs
            nea = (np.float32(DECAY) * eavg[cd]
                   + np.float32(1 - DECAY) * esum_cd[cd])
            outs["new_embed_avg"][cd] = nea
            sm = (ncs + np.float32(EPS)) * rfac
            outs["new_embed"][cd] = nea * (np.float32(1.0) / sm)
    outs["n_ambiguous"] = len(amb)
    outs["n_flips"] = len(flips)
    return outs


_last_exec_ns = None
_last_profile = None


def kernel(x, embed, embed_avg, cluster_size):
    global _last_exec_ns, _last_profile
    import os
    if "nc" not in _cached:
        _cached["nc"] = _build()
    nc = _cached["nc"]
    in_maps = _prep_inputs(x, embed, embed_avg, cluster_size)
    trace = bool(os.environ.get("VQ_TRACE"))
    res = bass_utils.run_bass_kernel_spmd(
        nc, in_maps, core_ids=list(range(NCORES)), trace=trace)
    _last_exec_ns = res.exec_time_ns
    _last_profile = res
    rs = res.results

    dist = np.concatenate([r["dist_o"] for r in rs], 0).reshape(1, NTOT, C)
    quant = np.concatenate([r["quant_o"] for r in rs], 0)
    ind = np.concatenate([r["ind_o"].reshape(NSH) for r in rs], 0)\
        .astype(np.int32)
    m8 = np.concatenate(
        [r["m8_o"].reshape(P, NT, 8).transpose(1, 0, 2).reshape(NSH, 8)
         for r in rs], 0)
    idx8 = np.concatenate(
        [r["idx8_o"].reshape(P, NT, 8).transpose(1, 0, 2).reshape(NSH, 8)
         for r in rs], 0)
    ne = np.concatenate([r["ne_o"].reshape(2 * P, D) for r in rs], 0)
    nea = np.concatenate([r["nea_o"].reshape(2 * P, D) for r in rs], 0)
    ncs = np.concatenate([np.ascontiguousarray(r["ncs_o"].T).reshape(2 * P)
                          for r in rs], 0)
    esr = np.concatenate([r["esr_o"] for r in rs], 0)   # (2048, 257)
    outs = {
        "quantize": quant,
        "embed_ind": ind,
        "dist": dist,
        "new_embed": ne,
        "new_cluster_size": ncs,
        "new_embed_avg": nea,
        "esum_red": esr,
        "m8": m8,
        "idx8": idx8,
    }
    _host_fixup(x, embed, cluster_size, embed_avg, outs)

    h, b, n, d = 1, 32, 2048, 256
    return (
        outs["quantize"].reshape(h, b, n, d),
        outs["embed_ind"].reshape(h, b, n),
        outs["dist"],
        outs["new_embed"].reshape(h, C, D),
        outs["new_cluster_size"].reshape(h, C),
        outs["new_embed_avg"].reshape(h, C, D),
    )
